# revision 1
# baseline (speedup 1.0000x reference)
"""Causal transformer block (B=2,S=2048,D=1024,H=16) on 8 trn2 NeuronCores.

Strategy: tensor-parallel attention over heads (2 heads/core) + token-parallel
MLP (512 tokens/core), glued by a single small AllToAll (2.1MB/core) that
re-shards the attention output from head-major to token-major.  LayerNorm
gains/biases are folded into the following matmul weights on the host, matmul
biases are folded in as rank-1 matmul accumulation steps, and softmax skips
the max-subtraction (scores are bounded) so each attention block is exactly
one QK^T matmul + one fused scale+exp activation + one PV matmul, with the
softmax denominator obtained from a ones-column appended to V.
"""
import numpy as np
from contextlib import ExitStack

import concourse.bass as bass
import concourse.bacc as bacc
import concourse.tile as tile
from concourse import mybir
from concourse.bass_utils import run_bass_kernel_spmd

f32 = mybir.dt.float32
f32r = mybir.dt.float32r
AF = mybir.ActivationFunctionType
ALU = mybir.AluOpType

B, S, D, H, HD, FF, NCORES = 2, 2048, 1024, 16, 64, 4096, 8
TOK = B * S            # 4096 total tokens
CHK = TOK // NCORES    # 512 tokens per core
D8 = D // 128          # 8 contraction tiles over D
NF = FF // 128         # 32 tiles over FF
NB = TOK // 512        # 8 token blocks of 512
EPS = 1e-5
SCALE = 1.0 / float(np.sqrt(HD))


def build(loops=1):
    nc = bacc.Bacc(None, num_devices=NCORES)

    x_h = nc.declare_dram_parameter("x", [TOK, D], f32, isOutput=False)
    xc_h = nc.declare_dram_parameter("xc", [CHK, D], f32, isOutput=False)
    wq_h = nc.declare_dram_parameter("wq", [D, 128], f32r, isOutput=False)
    wk_h = nc.declare_dram_parameter("wk", [D, 128], f32r, isOutput=False)
    wv_h = nc.declare_dram_parameter("wv", [D, 128], f32r, isOutput=False)
    bq_h = nc.declare_dram_parameter("bq", [1, 128], f32r, isOutput=False)
    bk_h = nc.declare_dram_parameter("bk", [1, 128], f32r, isOutput=False)
    bv_h = nc.declare_dram_parameter("bv", [1, 128], f32r, isOutput=False)
    wp_h = nc.declare_dram_parameter("wp", [D, D], f32r, isOutput=False)
    bp_h = nc.declare_dram_parameter("bp", [1, D], f32r, isOutput=False)
    w1_h = nc.declare_dram_parameter("w1", [D, FF], f32r, isOutput=False)
    b1_h = nc.declare_dram_parameter("b1", [1, FF], f32r, isOutput=False)
    w2_h = nc.declare_dram_parameter("w2", [FF, D], f32r, isOutput=False)
    b2_h = nc.declare_dram_parameter("b2", [1, D], f32r, isOutput=False)
    mask_h = nc.declare_dram_parameter("mask", [128, 896], f32, isOutput=False)
    ones_h = nc.declare_dram_parameter("ones", [1, 512], f32r, isOutput=False)
    onesv_h = nc.declare_dram_parameter("onesv", [128, 64], f32r, isOutput=False)
    id_h = nc.declare_dram_parameter("ident", [128, 128], f32, isOutput=False)
    out_h = nc.declare_dram_parameter("out", [CHK, D], f32, isOutput=True)

    with tile.TileContext(nc) as tc, ExitStack() as top:
        const = top.enter_context(tc.tile_pool(name="const", bufs=1))
        dram = top.enter_context(tc.tile_pool(name="dramp", bufs=1, space="DRAM"))

        ident = const.tile([128, 128], f32)
        nc.gpsimd.dma_start(out=ident[:], in_=id_h[:])
        mask = const.tile([128, 896], f32)
        nc.gpsimd.dma_start(out=mask[:], in_=mask_h[:])
        eps_sb = const.tile([128, 1], f32)
        nc.vector.memset(eps_sb[:], EPS)
        ones = const.tile([1, 512], f32r)
        nc.gpsimd.dma_start(out=ones[:], in_=ones_h[:])
        bq_sb = const.tile([1, 128], f32r)
        nc.gpsimd.dma_start(out=bq_sb[:], in_=bq_h[:])
        bk_sb = const.tile([1, 128], f32r)
        nc.gpsimd.dma_start(out=bk_sb[:], in_=bk_h[:])
        bv_sb = const.tile([1, 128], f32r)
        nc.gpsimd.dma_start(out=bv_sb[:], in_=bv_h[:])
        bp_sb = const.tile([1, D], f32r)
        nc.gpsimd.dma_start(out=bp_sb[:], in_=bp_h[:])
        b1_sb = const.tile([1, FF], f32r)
        nc.gpsimd.dma_start(out=b1_sb[:], in_=b1_h[:])
        b2_sb = const.tile([1, D], f32r)
        nc.gpsimd.dma_start(out=b2_sb[:], in_=b2_h[:])

        # AllToAll buffers: Ic[chunk, my-128-attn-rows, 512 toks] ->
        # Oc[dblock, 128 rows, my 512 toks]
        Ic = dram.tile([NCORES, 128, CHK], f32r)
        Oc = dram.tile([NCORES, 128, CHK], f32r)

        def one_pass():
            # ---------------- Phase A: LN1 + QKV(T) + attention -----------------
            with ExitStack() as A:
                wA = A.enter_context(tc.tile_pool(name="wA", bufs=1))
                wq_sb = wA.tile([128, D8, 128], f32r)
                nc.gpsimd.dma_start(out=wq_sb[:], in_=wq_h[:].rearrange("(a p) c -> p a c", p=128))
                wk_sb = wA.tile([128, D8, 128], f32r)
                nc.gpsimd.dma_start(out=wk_sb[:], in_=wk_h[:].rearrange("(a p) c -> p a c", p=128))
                wv_sb = wA.tile([128, D8, 128], f32r)
                nc.gpsimd.dma_start(out=wv_sb[:], in_=wv_h[:].rearrange("(a p) c -> p a c", p=128))

                QT = wA.tile([128, TOK], f32r)   # rows: (h2, hd)
                KT = wA.tile([128, TOK], f32r)
                V = wA.tile([128, TOK // 128, 2, HD + 1], f32r)  # tok-major V + ones col
                nc.gpsimd.dma_start(
                    out=V[:, :, :, HD:HD + 1],
                    in_=onesv_h[:].rearrange("p (g h o) -> p g h o", g=TOK // 128, h=2))

                lnp = A.enter_context(tc.tile_pool(name="lnp", bufs=3))
                hTp = A.enter_context(tc.tile_pool(name="hTp", bufs=2))
                vtp = A.enter_context(tc.tile_pool(name="vtp", bufs=2))
                ptp = A.enter_context(tc.tile_pool(name="ptp", bufs=8))
                aop = A.enter_context(tc.tile_pool(name="aop", bufs=3))
                smp = A.enter_context(tc.tile_pool(name="smp", bufs=4))
                ps_tr = A.enter_context(tc.tile_pool(name="ps_tr", bufs=2, space="PSUM"))
                ps_mm = A.enter_context(tc.tile_pool(name="ps_mm", bufs=2, space="PSUM"))
                ps_s = A.enter_context(tc.tile_pool(name="ps_s", bufs=2, space="PSUM"))
                ps_av = A.enter_context(tc.tile_pool(name="ps_av", bufs=2, space="PSUM"))

                for blk in range(NB):
                    hTb = hTp.tile([128, D8, 512], f32r, tag="hTb")
                    for t4 in range(4):
                        tt = blk * 4 + t4
                        xt = lnp.tile([128, D], f32, tag="xt")
                        nc.sync.dma_start(out=xt[:], in_=x_h[tt * 128:(tt + 1) * 128, :])
                        st = lnp.tile([128, 2, 6], f32, tag="st")
                        xv = xt[:].rearrange("p (s d) -> p s d", s=2)
                        nc.vector.bn_stats(out=st[:, 0, :], in_=xv[:, 0, :])
                        nc.vector.bn_stats(out=st[:, 1, :], in_=xv[:, 1, :])
                        mv = lnp.tile([128, 2], f32, tag="mv")
                        nc.vector.bn_aggr(out=mv[:], in_=st[:])
                        nc.scalar.activation(out=mv[:, 1:2], in_=mv[:, 1:2], func=AF.Sqrt,
                                             bias=eps_sb[:])
                        nc.vector.reciprocal(out=mv[:, 1:2], in_=mv[:, 1:2])
                        yt = lnp.tile([128, D], f32, tag="yt")
                        nc.gpsimd.tensor_scalar(out=yt[:], in0=xt[:],
                                                scalar1=mv[:, 0:1], scalar2=mv[:, 1:2],
                                                op0=ALU.subtract, op1=ALU.mult)
                        for half in range(2):
                            tp = ps_tr.tile([128, 512], f32, tag="tp")
                            for q in range(4):
                                d8 = half * 4 + q
                                nc.tensor.transpose(tp[:, q * 128:(q + 1) * 128],
                                                    yt[:, d8 * 128:(d8 + 1) * 128], ident[:])
                            nc.vector.tensor_copy(
                                out=hTb[:, half * 4:(half + 1) * 4, t4 * 128:(t4 + 1) * 128],
                                in_=tp[:].rearrange("p (q c) -> p q c", q=4))

                    for w_sb, b_sb, kind in ((wq_sb, bq_sb, "q"), (wk_sb, bk_sb, "k"),
                                             (wv_sb, bv_sb, "v")):
                        ps = ps_mm.tile([128, 512], f32, tag="qkvps")
                        for a in range(D8):
                            nc.tensor.matmul(ps[:], w_sb[:, a, :], hTb[:, a, :],
                                             start=(a == 0), stop=False)
                        nc.tensor.matmul(ps[:], b_sb[:], ones[:],
                                         start=False, stop=True)
                        if kind == "q":
                            nc.vector.tensor_copy(out=QT[:, blk * 512:(blk + 1) * 512], in_=ps[:])
                        elif kind == "k":
                            nc.vector.tensor_copy(out=KT[:, blk * 512:(blk + 1) * 512], in_=ps[:])
                        else:
                            vtmp = vtp.tile([128, 512], f32, tag="vtmp")
                            nc.vector.tensor_copy(out=vtmp[:], in_=ps[:])
                            for q in range(4):
                                tp2 = ps_tr.tile([128, 512], f32, tag="tp")
                                nc.tensor.transpose(tp2[:, 0:128],
                                                    vtmp[:, q * 128:(q + 1) * 128], ident[:])
                                nc.vector.tensor_copy(
                                    out=V[:, blk * 4 + q, :, 0:HD],
                                    in_=tp2[:, 0:128].rearrange("p (h d) -> p h d", h=2))

                # attention: 4 (batch, head) pairs
                for bh in range(B):
                    for h2 in range(2):
                        ro = h2 * HD
                        for j in range(4):          # query blocks of 512
                            q0 = bh * S + j * 512
                            nkt = 4 * (j + 1)
                            av = ps_av.tile([HD + 1, 512], f32, tag="av")
                            for kt in range(nkt):
                                k0 = bh * S + kt * 128
                                # straddling k-tiles: columns q < k0 are dead; only
                                # compute the live suffix and mask the one diagonal
                                # 128x128 sub-block.
                                ofs = max(0, 128 * kt - 512 * j)
                                w = 512 - ofs
                                sp = ps_s.tile([128, 512], f32, tag="sp")
                                nc.tensor.matmul(sp[:, 0:w], KT[ro:ro + HD, k0:k0 + 128],
                                                 QT[ro:ro + HD, q0 + ofs:q0 + 512],
                                                 start=True, stop=True)
                                if kt >= 4 * j:
                                    nc.vector.tensor_add(sp[:, 0:128], sp[:, 0:128],
                                                         mask[:, 384:512])
                                pt = ptp.tile([128, 512], f32r, tag="pt")
                                nc.scalar.activation(out=pt[:, 0:w], in_=sp[:, 0:w],
                                                     func=AF.Exp, scale=SCALE)
                                g = bh * (S // 128) + kt
                                nc.tensor.matmul(av[:, ofs:512], V[:, g, h2, :], pt[:, 0:w],
                                                 start=(kt == 0), stop=(kt == nkt - 1))
                            den = smp.tile([1, 512], f32r, tag="den")
                            nc.vector.tensor_copy(out=den[:], in_=av[HD:HD + 1, :])
                            with nc.allow_low_precision(reason="f32r rounding of softmax denom"):
                                nc.vector.reciprocal(out=den[:], in_=den[:])
                            bc = ps_s.tile([128, 512], f32, tag="sp")
                            nc.tensor.matmul(bc[0:HD, :], ones[:, 0:HD], den[:],
                                             start=True, stop=True)
                            rbc = aop.tile([HD, 512], f32, tag="rbc")
                            nc.scalar.activation(out=rbc[:], in_=bc[0:HD, :], func=AF.Copy)
                            ao = aop.tile([HD, 512], f32r, tag="ao")
                            nc.vector.tensor_mul(ao[:], av[0:HD, :], rbc[:])
                            chunk = bh * 4 + j
                            nc.sync.dma_start(out=Ic[chunk, h2 * HD:(h2 + 1) * HD, :],
                                              in_=ao[:])

            # ---------------- AllToAll: head-major -> token-major ---------------
            nc.gpsimd.collective_compute(
                "AllToAll", ALU.bypass,
                replica_groups=[list(range(NCORES))],
                ins=[Ic[:]], outs=[Oc[:]],
            )

            # ------------- Phase C: proj + residual + LN2 + MLP ------------------
            with ExitStack() as C:
                rB = C.enter_context(tc.tile_pool(name="rB", bufs=1))
                Oc_sb = rB.tile([128, NCORES, CHK], f32r)
                nc.sync.dma_start(out=Oc_sb[:], in_=Oc[:].rearrange("i p t -> p i t"))
                wp_sb = rB.tile([128, D8, D], f32r)
                nc.gpsimd.dma_start(out=wp_sb[:], in_=wp_h[:].rearrange("(a p) n -> p a n", p=128))
                x2_sb = rB.tile([128, 4, D], f32)
                y2T = rB.tile([128, D8, CHK], f32r)
                g1T = rB.tile([128, NF, CHK], f32r)

                with ExitStack() as C1:
                    lnp2 = C1.enter_context(tc.tile_pool(name="lnp2", bufs=2))
                    w1s = C1.enter_context(tc.tile_pool(name="w1s", bufs=2))
                    ps_p = C1.enter_context(tc.tile_pool(name="ps_p", bufs=2, space="PSUM"))
                    ps_t2 = C1.enter_context(tc.tile_pool(name="ps_t2", bufs=2, space="PSUM"))
                    ps_f1 = C1.enter_context(tc.tile_pool(name="ps_f1", bufs=2, space="PSUM"))

                    for t4 in range(4):
                        xct = lnp2.tile([128, D], f32, tag="xct")
                        nc.sync.dma_start(out=xct[:], in_=xc_h[t4 * 128:(t4 + 1) * 128, :])
                        for dc in range(2):
                            ps = ps_p.tile([128, 512], f32, tag="pp")
                            for a in range(D8):
                                nc.tensor.matmul(ps[:], Oc_sb[:, a, t4 * 128:(t4 + 1) * 128],
                                                 wp_sb[:, a, dc * 512:(dc + 1) * 512],
                                                 start=(a == 0), stop=False)
                            nc.tensor.matmul(ps[:], ones[:, 0:128],
                                             bp_sb[:, dc * 512:(dc + 1) * 512],
                                             start=False, stop=True)
                            nc.vector.tensor_add(x2_sb[:, t4, dc * 512:(dc + 1) * 512], ps[:],
                                                 xct[:, dc * 512:(dc + 1) * 512])
                        st2 = lnp2.tile([128, 2, 6], f32, tag="st2")
                        x2v = x2_sb[:, t4, :].rearrange("p (s d) -> p s d", s=2)
                        nc.vector.bn_stats(out=st2[:, 0, :], in_=x2v[:, 0, :])
                        nc.vector.bn_stats(out=st2[:, 1, :], in_=x2v[:, 1, :])
                        mv2 = lnp2.tile([128, 2], f32, tag="mv2")
                        nc.vector.bn_aggr(out=mv2[:], in_=st2[:])
                        nc.scalar.activation(out=mv2[:, 1:2], in_=mv2[:, 1:2], func=AF.Sqrt,
                                             bias=eps_sb[:])
                        nc.vector.reciprocal(out=mv2[:, 1:2], in_=mv2[:, 1:2])
                        y2 = lnp2.tile([128, D], f32, tag="y2")
                        nc.vector.tensor_scalar(out=y2[:], in0=x2_sb[:, t4, :],
                                                scalar1=mv2[:, 0:1], scalar2=mv2[:, 1:2],
                                                op0=ALU.subtract, op1=ALU.mult)
                        for half in range(2):
                            tp = ps_t2.tile([128, 512], f32, tag="t2")
                            for q in range(4):
                                d8 = half * 4 + q
                                nc.tensor.transpose(tp[:, q * 128:(q + 1) * 128],
                                                    y2[:, d8 * 128:(d8 + 1) * 128], ident[:])
                            nc.vector.tensor_copy(
                                out=y2T[:, half * 4:(half + 1) * 4, t4 * 128:(t4 + 1) * 128],
                                in_=tp[:].rearrange("p (q c) -> p q c", q=4))

                    # fc1 + gelu -> g1T resident
                    for ff in range(NF):
                        w1t = w1s.tile([128, D8, 128], f32r, tag="w1t")
                        nc.gpsimd.dma_start(
                            out=w1t[:],
                            in_=w1_h[:, ff * 128:(ff + 1) * 128].rearrange("(a p) c -> p a c", p=128))
                        ps = ps_f1.tile([128, 512], f32, tag="f1")
                        for a in range(D8):
                            nc.tensor.matmul(ps[:], w1t[:, a, :], y2T[:, a, :],
                                             start=(a == 0), stop=False)
                        nc.tensor.matmul(ps[:], b1_sb[:, ff * 128:(ff + 1) * 128], ones[:],
                                         start=False, stop=True)
                        nc.scalar.activation(out=g1T[:, ff, :], in_=ps[:], func=AF.Gelu)

                # fc2: 8 psum accumulators (4 token tiles x 2 column halves)
                ps_f2 = C.enter_context(tc.tile_pool(name="ps_f2", bufs=1, space="PSUM"))
                w2s = C.enter_context(tc.tile_pool(name="w2s", bufs=3))
                outp = C.enter_context(tc.tile_pool(name="outp", bufs=2))
                accs = [ps_f2.tile([128, 512], f32, name=f"acc{i}", tag=f"acc{i}")
                        for i in range(8)]
                for ff in range(NF):
                    w2t = w2s.tile([128, D], f32r, tag="w2t")
                    nc.gpsimd.dma_start(out=w2t[:], in_=w2_h[ff * 128:(ff + 1) * 128, :])
                    for t4 in range(4):
                        for dc in range(2):
                            nc.tensor.matmul(accs[t4 * 2 + dc][:],
                                             g1T[:, ff, t4 * 128:(t4 + 1) * 128],
                                             w2t[:, dc * 512:(dc + 1) * 512],
                                             start=(ff == 0), stop=False)
                for t4 in range(4):
                    ot = outp.tile([128, D], f32, tag="ot")
                    for dc in range(2):
                        i = t4 * 2 + dc
                        nc.tensor.matmul(accs[i][:], ones[:, 0:128],
                                         b2_sb[:, dc * 512:(dc + 1) * 512],
                                         start=False, stop=True)
                        nc.vector.tensor_add(ot[:, dc * 512:(dc + 1) * 512], accs[i][:],
                                             x2_sb[:, t4, dc * 512:(dc + 1) * 512])
                    nc.sync.dma_start(out=out_h[t4 * 128:(t4 + 1) * 128, :], in_=ot[:])


        for _ in range(loops):
            one_pass()
    nc.finalize()
    return nc


_NC_CACHE = []
LAST = None


def _get_nc():
    if not _NC_CACHE:
        _NC_CACHE.append(build())
    return _NC_CACHE[0]


def prepare_in_maps(inputs):
    f = np.float32
    x = np.ascontiguousarray(np.asarray(inputs["x"], f).reshape(TOK, D))
    ln1_g = np.asarray(inputs["ln1_g"], np.float64)
    ln1_b = np.asarray(inputs["ln1_b"], np.float64)
    ln2_g = np.asarray(inputs["ln2_g"], np.float64)
    ln2_b = np.asarray(inputs["ln2_b"], np.float64)
    w_qkv = np.asarray(inputs["w_qkv"], np.float64)
    b_qkv = np.asarray(inputs["b_qkv"], np.float64)
    w_fc1 = np.asarray(inputs["w_fc1"], np.float64)
    b_fc1 = np.asarray(inputs["b_fc1"], np.float64)

    w_eff = (w_qkv * ln1_g[:, None]).astype(f)
    b_eff = (b_qkv + ln1_b @ w_qkv).astype(f)
    w1_eff = (w_fc1 * ln2_g[:, None]).astype(f)
    b1_eff = (b_fc1 + ln2_b @ w_fc1).astype(f)
    wp = np.ascontiguousarray(np.asarray(inputs["w_proj"], f))
    bp = np.asarray(inputs["b_proj"], f).reshape(1, D)
    w2 = np.ascontiguousarray(np.asarray(inputs["w_fc2"], f))
    b2 = np.asarray(inputs["b_fc2"], f).reshape(1, D)

    # additive causal mask: S[i, j] = mask[i, (384 - a) + j] is 0 where token
    # (q0 + j) >= (k0 + i), -1e9 (-> exp 0) otherwise
    mask = np.full((128, 896), -1e9, f)
    for i in range(128):
        mask[i, i + 384:] = 0.0
    ident = np.eye(128, dtype=f)

    in_maps = []
    for c in range(NCORES):
        cs = slice(128 * c, 128 * (c + 1))
        in_maps.append({
            "x": x,
            "xc": np.ascontiguousarray(x[CHK * c:CHK * (c + 1)]),
            "wq": np.ascontiguousarray(w_eff[:, 0 * D:1 * D][:, cs]),
            "wk": np.ascontiguousarray(w_eff[:, 1 * D:2 * D][:, cs]),
            "wv": np.ascontiguousarray(w_eff[:, 2 * D:3 * D][:, cs]),
            "bq": np.ascontiguousarray(b_eff[None, 0 * D:1 * D][:, cs]),
            "bk": np.ascontiguousarray(b_eff[None, 1 * D:2 * D][:, cs]),
            "bv": np.ascontiguousarray(b_eff[None, 2 * D:3 * D][:, cs]),
            "wp": wp, "bp": bp,
            "w1": w1_eff, "b1": b1_eff.reshape(1, FF),
            "w2": w2, "b2": b2,
            "mask": mask, "ident": ident,
            "ones": np.ones((1, 512), f),
            "onesv": np.ones((128, 64), f),
        })
    return in_maps


def kernel(**inputs):
    global LAST
    in_maps = prepare_in_maps(inputs)
    nc = _get_nc()
    res = run_bass_kernel_spmd(nc, in_maps, list(range(NCORES)))
    LAST = res
    out = np.concatenate([res.results[c]["out"] for c in range(NCORES)], axis=0)
    return out.reshape(B, S, D).astype(np.float32, copy=False)



# revision 2
# speedup vs baseline: 1.4399x; 1.4399x over previous
"""Causal transformer block (B=2,S=2048,D=1024,H=16) on 8 trn2 NeuronCores.

Strategy: tensor-parallel attention over heads (2 heads/core) + token-parallel
MLP (512 tokens/core), glued by a single small AllToAll (2.1MB/core) that
re-shards the attention output from head-major to token-major.  LayerNorm
gains/biases are folded into the following matmul weights on the host, matmul
biases are folded in as rank-1 matmul accumulation steps, and softmax skips
the max-subtraction (scores are bounded) so each attention block is exactly
one QK^T matmul + one fused scale+exp activation + one PV matmul, with the
softmax denominator obtained from a ones-column appended to V.
"""
import numpy as np
from contextlib import ExitStack

import concourse.bass as bass
import concourse.bacc as bacc
import concourse.tile as tile
from concourse import mybir
from concourse.bass_utils import run_bass_kernel_spmd

f32 = mybir.dt.float32
f32r = mybir.dt.float32r
AF = mybir.ActivationFunctionType
ALU = mybir.AluOpType

B, S, D, H, HD, FF, NCORES = 2, 2048, 1024, 16, 64, 4096, 8
TOK = B * S            # 4096 total tokens
CHK = TOK // NCORES    # 512 tokens per core
D8 = D // 128          # 8 contraction tiles over D
NF = FF // 128         # 32 tiles over FF
NB = TOK // 512        # 8 token blocks of 512
EPS = 1e-5
SCALE = 1.0 / float(np.sqrt(HD))


def build(loops=1):
    nc = bacc.Bacc(None, num_devices=NCORES)

    x_h = nc.declare_dram_parameter("x", [TOK, D], f32, isOutput=False)
    xc_h = nc.declare_dram_parameter("xc", [CHK, D], f32, isOutput=False)
    wq_h = nc.declare_dram_parameter("wq", [D, 128], f32r, isOutput=False)
    wk_h = nc.declare_dram_parameter("wk", [D, 128], f32r, isOutput=False)
    wv_h = nc.declare_dram_parameter("wv", [D, 128], f32r, isOutput=False)
    bq_h = nc.declare_dram_parameter("bq", [1, 128], f32r, isOutput=False)
    bk_h = nc.declare_dram_parameter("bk", [1, 128], f32r, isOutput=False)
    bv_h = nc.declare_dram_parameter("bv", [1, 128], f32r, isOutput=False)
    wp_h = nc.declare_dram_parameter("wp", [D, D], f32r, isOutput=False)
    bp_h = nc.declare_dram_parameter("bp", [1, D], f32r, isOutput=False)
    w1_h = nc.declare_dram_parameter("w1", [D, FF], f32r, isOutput=False)
    b1_h = nc.declare_dram_parameter("b1", [1, FF], f32r, isOutput=False)
    w2_h = nc.declare_dram_parameter("w2", [FF, D], f32r, isOutput=False)
    b2_h = nc.declare_dram_parameter("b2", [1, D], f32r, isOutput=False)
    mask_h = nc.declare_dram_parameter("mask", [128, 896], f32, isOutput=False)
    ones_h = nc.declare_dram_parameter("ones", [1, 512], f32r, isOutput=False)
    onesv_h = nc.declare_dram_parameter("onesv", [128, 64], f32r, isOutput=False)
    id_h = nc.declare_dram_parameter("ident", [128, 128], f32, isOutput=False)
    out_h = nc.declare_dram_parameter("out", [CHK, D], f32, isOutput=True)

    with tile.TileContext(nc) as tc, ExitStack() as top:
        const = top.enter_context(tc.tile_pool(name="const", bufs=1))
        dram = top.enter_context(tc.tile_pool(name="dramp", bufs=1, space="DRAM"))

        ident = const.tile([128, 128], f32)
        nc.gpsimd.dma_start(out=ident[:], in_=id_h[:])
        mask = const.tile([128, 896], f32)
        nc.gpsimd.dma_start(out=mask[:], in_=mask_h[:])
        eps_sb = const.tile([128, 1], f32)
        nc.vector.memset(eps_sb[:], EPS)
        ones = const.tile([1, 512], f32r)
        nc.gpsimd.dma_start(out=ones[:], in_=ones_h[:])
        bq_sb = const.tile([1, 128], f32r)
        nc.gpsimd.dma_start(out=bq_sb[:], in_=bq_h[:])
        bk_sb = const.tile([1, 128], f32r)
        nc.gpsimd.dma_start(out=bk_sb[:], in_=bk_h[:])
        bv_sb = const.tile([1, 128], f32r)
        nc.gpsimd.dma_start(out=bv_sb[:], in_=bv_h[:])
        bp_sb = const.tile([1, D], f32r)
        nc.gpsimd.dma_start(out=bp_sb[:], in_=bp_h[:])
        b1_sb = const.tile([1, FF], f32r)
        nc.gpsimd.dma_start(out=b1_sb[:], in_=b1_h[:])
        b2_sb = const.tile([1, D], f32r)
        nc.gpsimd.dma_start(out=b2_sb[:], in_=b2_h[:])

        # AllToAll buffers: Ic[chunk, my-128-attn-rows, 512 toks] ->
        # Oc[dblock, 128 rows, my 512 toks]
        Ic = dram.tile([NCORES, 128, CHK], f32r)
        Oc = dram.tile([NCORES, 128, CHK], f32r)

        def one_pass():
            # ---------------- Phase A: LN1 + QKV(T) + attention -----------------
            with ExitStack() as A:
                wA = A.enter_context(tc.tile_pool(name="wA", bufs=1))
                wq_sb = wA.tile([128, D8, 128], f32r)
                nc.gpsimd.dma_start(out=wq_sb[:], in_=wq_h[:].rearrange("(a p) c -> p a c", p=128))
                wk_sb = wA.tile([128, D8, 128], f32r)
                nc.gpsimd.dma_start(out=wk_sb[:], in_=wk_h[:].rearrange("(a p) c -> p a c", p=128))
                wv_sb = wA.tile([128, D8, 128], f32r)
                nc.gpsimd.dma_start(out=wv_sb[:], in_=wv_h[:].rearrange("(a p) c -> p a c", p=128))

                QT = wA.tile([128, TOK], f32r)   # rows: (h2, hd)
                KT = wA.tile([128, TOK], f32r)
                V = wA.tile([128, TOK // 128, 2, HD + 1], f32r)  # tok-major V + ones col
                nc.gpsimd.dma_start(
                    out=V[:, :, :, HD:HD + 1],
                    in_=onesv_h[:].rearrange("p (g h o) -> p g h o", g=TOK // 128, h=2))

                lnp = A.enter_context(tc.tile_pool(name="lnp", bufs=3))
                hTp = A.enter_context(tc.tile_pool(name="hTp", bufs=2))
                vtp = A.enter_context(tc.tile_pool(name="vtp", bufs=2))
                ptp = A.enter_context(tc.tile_pool(name="ptp", bufs=8))
                aop = A.enter_context(tc.tile_pool(name="aop", bufs=3))
                smp = A.enter_context(tc.tile_pool(name="smp", bufs=4))
                ps_tr = A.enter_context(tc.tile_pool(name="ps_tr", bufs=2, space="PSUM"))
                ps_mm = A.enter_context(tc.tile_pool(name="ps_mm", bufs=2, space="PSUM"))
                ps_s = A.enter_context(tc.tile_pool(name="ps_s", bufs=2, space="PSUM"))
                ps_av = A.enter_context(tc.tile_pool(name="ps_av", bufs=2, space="PSUM"))

                for blk in range(NB):
                    hTb = hTp.tile([128, D8, 512], f32r, tag="hTb")
                    for t4 in range(4):
                        tt = blk * 4 + t4
                        xt = lnp.tile([128, D], f32, tag="xt")
                        nc.sync.dma_start(out=xt[:], in_=x_h[tt * 128:(tt + 1) * 128, :])
                        st = lnp.tile([128, 2, 6], f32, tag="st")
                        xv = xt[:].rearrange("p (s d) -> p s d", s=2)
                        nc.vector.bn_stats(out=st[:, 0, :], in_=xv[:, 0, :])
                        nc.vector.bn_stats(out=st[:, 1, :], in_=xv[:, 1, :])
                        mv = lnp.tile([128, 2], f32, tag="mv")
                        nc.vector.bn_aggr(out=mv[:], in_=st[:])
                        nc.scalar.activation(out=mv[:, 1:2], in_=mv[:, 1:2], func=AF.Sqrt,
                                             bias=eps_sb[:])
                        nc.vector.reciprocal(out=mv[:, 1:2], in_=mv[:, 1:2])
                        yt = lnp.tile([128, D], f32, tag="yt")
                        nc.vector.tensor_scalar(out=yt[:], in0=xt[:],
                                                scalar1=mv[:, 0:1], scalar2=mv[:, 1:2],
                                                op0=ALU.subtract, op1=ALU.mult)
                        for half in range(2):
                            tp = ps_tr.tile([128, 512], f32, tag="tp")
                            for q in range(4):
                                d8 = half * 4 + q
                                nc.tensor.transpose(tp[:, q * 128:(q + 1) * 128],
                                                    yt[:, d8 * 128:(d8 + 1) * 128], ident[:])
                            nc.vector.tensor_copy(
                                out=hTb[:, half * 4:(half + 1) * 4, t4 * 128:(t4 + 1) * 128],
                                in_=tp[:].rearrange("p (q c) -> p q c", q=4))

                    for w_sb, b_sb, kind in ((wq_sb, bq_sb, "q"), (wk_sb, bk_sb, "k"),
                                             (wv_sb, bv_sb, "v")):
                        ps = ps_mm.tile([128, 512], f32, tag="qkvps")
                        for a in range(D8):
                            nc.tensor.matmul(ps[:], w_sb[:, a, :], hTb[:, a, :],
                                             start=(a == 0), stop=False)
                        nc.tensor.matmul(ps[:], b_sb[:], ones[:],
                                         start=False, stop=True)
                        if kind == "q":
                            nc.vector.tensor_copy(out=QT[:, blk * 512:(blk + 1) * 512], in_=ps[:])
                        elif kind == "k":
                            nc.vector.tensor_copy(out=KT[:, blk * 512:(blk + 1) * 512], in_=ps[:])
                        else:
                            vtmp = vtp.tile([128, 512], f32, tag="vtmp")
                            nc.vector.tensor_copy(out=vtmp[:], in_=ps[:])
                            for q in range(4):
                                tp2 = ps_tr.tile([128, 512], f32, tag="tp")
                                nc.tensor.transpose(tp2[:, 0:128],
                                                    vtmp[:, q * 128:(q + 1) * 128], ident[:])
                                nc.vector.tensor_copy(
                                    out=V[:, blk * 4 + q, :, 0:HD],
                                    in_=tp2[:, 0:128].rearrange("p (h d) -> p h d", h=2))

                # attention: 4 (batch, head) pairs
                for bh in range(B):
                    for h2 in range(2):
                        ro = h2 * HD
                        for j in range(4):          # query blocks of 512
                            q0 = bh * S + j * 512
                            nkt = 4 * (j + 1)
                            av = ps_av.tile([HD + 1, 512], f32, tag="av")
                            for kt in range(nkt):
                                k0 = bh * S + kt * 128
                                # straddling k-tiles: columns q < k0 are dead; only
                                # compute the live suffix and mask the one diagonal
                                # 128x128 sub-block.
                                ofs = max(0, 128 * kt - 512 * j)
                                w = 512 - ofs
                                sp = ps_s.tile([128, 512], f32, tag="sp")
                                nc.tensor.matmul(sp[:, 0:w], KT[ro:ro + HD, k0:k0 + 128],
                                                 QT[ro:ro + HD, q0 + ofs:q0 + 512],
                                                 start=True, stop=True)
                                if kt >= 4 * j:
                                    nc.vector.tensor_add(sp[:, 0:128], sp[:, 0:128],
                                                         mask[:, 384:512])
                                pt = ptp.tile([128, 512], f32r, tag="pt")
                                nc.scalar.activation(out=pt[:, 0:w], in_=sp[:, 0:w],
                                                     func=AF.Exp, scale=SCALE)
                                g = bh * (S // 128) + kt
                                nc.tensor.matmul(av[:, ofs:512], V[:, g, h2, :], pt[:, 0:w],
                                                 start=(kt == 0), stop=(kt == nkt - 1))
                            den = smp.tile([1, 512], f32r, tag="den")
                            nc.vector.tensor_copy(out=den[:], in_=av[HD:HD + 1, :])
                            with nc.allow_low_precision(reason="f32r rounding of softmax denom"):
                                nc.vector.reciprocal(out=den[:], in_=den[:])
                            bc = ps_s.tile([128, 512], f32, tag="sp")
                            nc.tensor.matmul(bc[0:HD, :], ones[:, 0:HD], den[:],
                                             start=True, stop=True)
                            rbc = aop.tile([HD, 512], f32, tag="rbc")
                            nc.scalar.activation(out=rbc[:], in_=bc[0:HD, :], func=AF.Copy)
                            ao = aop.tile([HD, 512], f32r, tag="ao")
                            nc.vector.tensor_mul(ao[:], av[0:HD, :], rbc[:])
                            chunk = bh * 4 + j
                            nc.sync.dma_start(out=Ic[chunk, h2 * HD:(h2 + 1) * HD, :],
                                              in_=ao[:])

            # ---------------- AllToAll: head-major -> token-major ---------------
            nc.gpsimd.collective_compute(
                "AllToAll", ALU.bypass,
                replica_groups=[list(range(NCORES))],
                ins=[Ic[:]], outs=[Oc[:]],
            )

            # ------------- Phase C: proj + residual + LN2 + MLP ------------------
            with ExitStack() as C:
                rB = C.enter_context(tc.tile_pool(name="rB", bufs=1))
                Oc_sb = rB.tile([128, NCORES, CHK], f32r)
                nc.sync.dma_start(out=Oc_sb[:], in_=Oc[:].rearrange("i p t -> p i t"))
                wp_sb = rB.tile([128, D8, D], f32r)
                nc.gpsimd.dma_start(out=wp_sb[:], in_=wp_h[:].rearrange("(a p) n -> p a n", p=128))
                x2_sb = rB.tile([128, 4, D], f32)
                y2T = rB.tile([128, D8, CHK], f32r)
                g1T = rB.tile([128, NF, CHK], f32r)

                with ExitStack() as C1:
                    lnp2 = C1.enter_context(tc.tile_pool(name="lnp2", bufs=2))
                    w1s = C1.enter_context(tc.tile_pool(name="w1s", bufs=2))
                    ps_p = C1.enter_context(tc.tile_pool(name="ps_p", bufs=2, space="PSUM"))
                    ps_t2 = C1.enter_context(tc.tile_pool(name="ps_t2", bufs=2, space="PSUM"))
                    ps_f1 = C1.enter_context(tc.tile_pool(name="ps_f1", bufs=2, space="PSUM"))

                    for t4 in range(4):
                        xct = lnp2.tile([128, D], f32, tag="xct")
                        nc.sync.dma_start(out=xct[:], in_=xc_h[t4 * 128:(t4 + 1) * 128, :])
                        for dc in range(2):
                            ps = ps_p.tile([128, 512], f32, tag="pp")
                            for a in range(D8):
                                nc.tensor.matmul(ps[:], Oc_sb[:, a, t4 * 128:(t4 + 1) * 128],
                                                 wp_sb[:, a, dc * 512:(dc + 1) * 512],
                                                 start=(a == 0), stop=False)
                            nc.tensor.matmul(ps[:], ones[:, 0:128],
                                             bp_sb[:, dc * 512:(dc + 1) * 512],
                                             start=False, stop=True)
                            nc.vector.tensor_add(x2_sb[:, t4, dc * 512:(dc + 1) * 512], ps[:],
                                                 xct[:, dc * 512:(dc + 1) * 512])
                        st2 = lnp2.tile([128, 2, 6], f32, tag="st2")
                        x2v = x2_sb[:, t4, :].rearrange("p (s d) -> p s d", s=2)
                        nc.vector.bn_stats(out=st2[:, 0, :], in_=x2v[:, 0, :])
                        nc.vector.bn_stats(out=st2[:, 1, :], in_=x2v[:, 1, :])
                        mv2 = lnp2.tile([128, 2], f32, tag="mv2")
                        nc.vector.bn_aggr(out=mv2[:], in_=st2[:])
                        nc.scalar.activation(out=mv2[:, 1:2], in_=mv2[:, 1:2], func=AF.Sqrt,
                                             bias=eps_sb[:])
                        nc.vector.reciprocal(out=mv2[:, 1:2], in_=mv2[:, 1:2])
                        y2 = lnp2.tile([128, D], f32, tag="y2")
                        nc.vector.tensor_scalar(out=y2[:], in0=x2_sb[:, t4, :],
                                                scalar1=mv2[:, 0:1], scalar2=mv2[:, 1:2],
                                                op0=ALU.subtract, op1=ALU.mult)
                        for half in range(2):
                            tp = ps_t2.tile([128, 512], f32, tag="t2")
                            for q in range(4):
                                d8 = half * 4 + q
                                nc.tensor.transpose(tp[:, q * 128:(q + 1) * 128],
                                                    y2[:, d8 * 128:(d8 + 1) * 128], ident[:])
                            nc.vector.tensor_copy(
                                out=y2T[:, half * 4:(half + 1) * 4, t4 * 128:(t4 + 1) * 128],
                                in_=tp[:].rearrange("p (q c) -> p q c", q=4))

                    # fc1 + gelu -> g1T resident
                    for ff in range(NF):
                        w1t = w1s.tile([128, D8, 128], f32r, tag="w1t")
                        nc.gpsimd.dma_start(
                            out=w1t[:],
                            in_=w1_h[:, ff * 128:(ff + 1) * 128].rearrange("(a p) c -> p a c", p=128))
                        ps = ps_f1.tile([128, 512], f32, tag="f1")
                        for a in range(D8):
                            nc.tensor.matmul(ps[:], w1t[:, a, :], y2T[:, a, :],
                                             start=(a == 0), stop=False)
                        nc.tensor.matmul(ps[:], b1_sb[:, ff * 128:(ff + 1) * 128], ones[:],
                                         start=False, stop=True)
                        nc.scalar.activation(out=g1T[:, ff, :], in_=ps[:], func=AF.Gelu)

                # fc2: 8 psum accumulators (4 token tiles x 2 column halves)
                ps_f2 = C.enter_context(tc.tile_pool(name="ps_f2", bufs=1, space="PSUM"))
                w2s = C.enter_context(tc.tile_pool(name="w2s", bufs=3))
                outp = C.enter_context(tc.tile_pool(name="outp", bufs=2))
                accs = [ps_f2.tile([128, 512], f32, name=f"acc{i}", tag=f"acc{i}")
                        for i in range(8)]
                for ff in range(NF):
                    w2t = w2s.tile([128, D], f32r, tag="w2t")
                    nc.gpsimd.dma_start(out=w2t[:], in_=w2_h[ff * 128:(ff + 1) * 128, :])
                    for t4 in range(4):
                        for dc in range(2):
                            nc.tensor.matmul(accs[t4 * 2 + dc][:],
                                             g1T[:, ff, t4 * 128:(t4 + 1) * 128],
                                             w2t[:, dc * 512:(dc + 1) * 512],
                                             start=(ff == 0), stop=False)
                for t4 in range(4):
                    ot = outp.tile([128, D], f32, tag="ot")
                    for dc in range(2):
                        i = t4 * 2 + dc
                        nc.tensor.matmul(accs[i][:], ones[:, 0:128],
                                         b2_sb[:, dc * 512:(dc + 1) * 512],
                                         start=False, stop=True)
                        nc.vector.tensor_add(ot[:, dc * 512:(dc + 1) * 512], accs[i][:],
                                             x2_sb[:, t4, dc * 512:(dc + 1) * 512])
                    nc.sync.dma_start(out=out_h[t4 * 128:(t4 + 1) * 128, :], in_=ot[:])


        for _ in range(loops):
            one_pass()
    nc.finalize()
    return nc


_NC_CACHE = []
LAST = None


def _get_nc():
    if not _NC_CACHE:
        _NC_CACHE.append(build())
    return _NC_CACHE[0]


def prepare_in_maps(inputs):
    f = np.float32
    x = np.ascontiguousarray(np.asarray(inputs["x"], f).reshape(TOK, D))
    ln1_g = np.asarray(inputs["ln1_g"], np.float64)
    ln1_b = np.asarray(inputs["ln1_b"], np.float64)
    ln2_g = np.asarray(inputs["ln2_g"], np.float64)
    ln2_b = np.asarray(inputs["ln2_b"], np.float64)
    w_qkv = np.asarray(inputs["w_qkv"], np.float64)
    b_qkv = np.asarray(inputs["b_qkv"], np.float64)
    w_fc1 = np.asarray(inputs["w_fc1"], np.float64)
    b_fc1 = np.asarray(inputs["b_fc1"], np.float64)

    w_eff = (w_qkv * ln1_g[:, None]).astype(f)
    b_eff = (b_qkv + ln1_b @ w_qkv).astype(f)
    w1_eff = (w_fc1 * ln2_g[:, None]).astype(f)
    b1_eff = (b_fc1 + ln2_b @ w_fc1).astype(f)
    wp = np.ascontiguousarray(np.asarray(inputs["w_proj"], f))
    bp = np.asarray(inputs["b_proj"], f).reshape(1, D)
    w2 = np.ascontiguousarray(np.asarray(inputs["w_fc2"], f))
    b2 = np.asarray(inputs["b_fc2"], f).reshape(1, D)

    # additive causal mask: S[i, j] = mask[i, (384 - a) + j] is 0 where token
    # (q0 + j) >= (k0 + i), -1e9 (-> exp 0) otherwise
    mask = np.full((128, 896), -1e9, f)
    for i in range(128):
        mask[i, i + 384:] = 0.0
    ident = np.eye(128, dtype=f)

    in_maps = []
    for c in range(NCORES):
        cs = slice(128 * c, 128 * (c + 1))
        in_maps.append({
            "x": x,
            "xc": np.ascontiguousarray(x[CHK * c:CHK * (c + 1)]),
            "wq": np.ascontiguousarray(w_eff[:, 0 * D:1 * D][:, cs]),
            "wk": np.ascontiguousarray(w_eff[:, 1 * D:2 * D][:, cs]),
            "wv": np.ascontiguousarray(w_eff[:, 2 * D:3 * D][:, cs]),
            "bq": np.ascontiguousarray(b_eff[None, 0 * D:1 * D][:, cs]),
            "bk": np.ascontiguousarray(b_eff[None, 1 * D:2 * D][:, cs]),
            "bv": np.ascontiguousarray(b_eff[None, 2 * D:3 * D][:, cs]),
            "wp": wp, "bp": bp,
            "w1": w1_eff, "b1": b1_eff.reshape(1, FF),
            "w2": w2, "b2": b2,
            "mask": mask, "ident": ident,
            "ones": np.ones((1, 512), f),
            "onesv": np.ones((128, 64), f),
        })
    return in_maps


def kernel(**inputs):
    global LAST
    in_maps = prepare_in_maps(inputs)
    nc = _get_nc()
    res = run_bass_kernel_spmd(nc, in_maps, list(range(NCORES)))
    LAST = res
    out = np.concatenate([res.results[c]["out"] for c in range(NCORES)], axis=0)
    return out.reshape(B, S, D).astype(np.float32, copy=False)



# revision 3
# speedup vs baseline: 1.7012x; 1.1815x over previous
"""Causal transformer block (B=2,S=2048,D=1024,H=16) on 8 trn2 NeuronCores.

Strategy: tensor-parallel attention over heads (2 heads/core) + token-parallel
MLP (512 tokens/core), glued by a single small AllToAll (1MB/core, bf16) that
re-shards the attention output from head-major to token-major.  LayerNorm
gains/biases are folded into the following matmul weights on the host; QKV /
fc1 biases are folded into the PSUM->SBUF copy as per-partition activation
biases; softmax skips the max-subtraction (scores are bounded) and gets its
denominator from a ones-column appended to V.  All matmul operands are bf16
(f32 PSUM accumulation); the residual spine stays f32.  Activation transposes
(x -> hT, y2 -> y2T) run on the DMA XBAR (dma_start_transpose) instead of the
PE array; V's transpose stays on the PE because its destination interleaves
the ones column.
"""
import numpy as np
import ml_dtypes
from contextlib import ExitStack

import concourse.bass as bass
import concourse.bacc as bacc
import concourse.tile as tile
from concourse import mybir
from concourse.bass_utils import run_bass_kernel_spmd

f32 = mybir.dt.float32
f32r = mybir.dt.float32r
bf16 = mybir.dt.bfloat16
AF = mybir.ActivationFunctionType
ALU = mybir.AluOpType

B, S, D, H, HD, FF, NCORES = 2, 2048, 1024, 16, 64, 4096, 8
TOK = B * S            # 4096 total tokens
CHK = TOK // NCORES    # 512 tokens per core
D8 = D // 128          # 8 contraction tiles over D
NF = FF // 128         # 32 tiles over FF
NB = TOK // 512        # 8 token blocks of 512
EPS = 1e-5
SCALE = 1.0 / float(np.sqrt(HD))
NPBF = ml_dtypes.bfloat16


def build(loops=1):
    nc = bacc.Bacc(None, num_devices=NCORES)

    xbf_h = nc.declare_dram_parameter("xbf", [TOK, D], bf16, isOutput=False)
    xc_h = nc.declare_dram_parameter("xc", [CHK, D], f32, isOutput=False)
    wq_h = nc.declare_dram_parameter("wq", [128, D8, 128], bf16, isOutput=False)
    wk_h = nc.declare_dram_parameter("wk", [128, D8, 128], bf16, isOutput=False)
    wv_h = nc.declare_dram_parameter("wv", [128, D8, 128], bf16, isOutput=False)
    bq_h = nc.declare_dram_parameter("bq", [128, 1], f32, isOutput=False)
    bk_h = nc.declare_dram_parameter("bk", [128, 1], f32, isOutput=False)
    bv_h = nc.declare_dram_parameter("bv", [128, 1], f32, isOutput=False)
    wp_h = nc.declare_dram_parameter("wp", [128, D8, D], bf16, isOutput=False)
    bp_h = nc.declare_dram_parameter("bp", [1, D], f32r, isOutput=False)
    w1_h = nc.declare_dram_parameter("w1", [128, NF, D8, 128], bf16, isOutput=False)
    b1_h = nc.declare_dram_parameter("b1", [128, NF], f32, isOutput=False)
    w2_h = nc.declare_dram_parameter("w2", [FF, D], bf16, isOutput=False)
    b2_h = nc.declare_dram_parameter("b2", [1, D], f32r, isOutput=False)
    mask_h = nc.declare_dram_parameter("mask", [128, 128], f32, isOutput=False)
    ones_h = nc.declare_dram_parameter("ones", [1, 512], f32r, isOutput=False)
    id_h = nc.declare_dram_parameter("ident", [128, 128], bf16, isOutput=False)
    out_h = nc.declare_dram_parameter("out", [CHK, D], f32, isOutput=True)

    with tile.TileContext(nc) as tc, ExitStack() as top:
        const = top.enter_context(tc.tile_pool(name="const", bufs=1))
        dram = top.enter_context(tc.tile_pool(name="dramp", bufs=1, space="DRAM"))

        ident = const.tile([128, 128], bf16)
        nc.gpsimd.dma_start(out=ident[:], in_=id_h[:])
        mask = const.tile([128, 128], f32)
        nc.gpsimd.dma_start(out=mask[:], in_=mask_h[:])
        eps_sb = const.tile([128, 1], f32)
        nc.vector.memset(eps_sb[:], EPS)
        ones = const.tile([1, 512], f32r)
        nc.gpsimd.dma_start(out=ones[:], in_=ones_h[:])
        bq_sb = const.tile([128, 1], f32)
        nc.gpsimd.dma_start(out=bq_sb[:], in_=bq_h[:])
        bk_sb = const.tile([128, 1], f32)
        nc.gpsimd.dma_start(out=bk_sb[:], in_=bk_h[:])
        bv_sb = const.tile([128, 1], f32)
        nc.gpsimd.dma_start(out=bv_sb[:], in_=bv_h[:])
        bp_sb = const.tile([1, D], f32r)
        nc.gpsimd.dma_start(out=bp_sb[:], in_=bp_h[:])
        b1_sb = const.tile([128, NF], f32)
        nc.gpsimd.dma_start(out=b1_sb[:], in_=b1_h[:])
        b2_sb = const.tile([1, D], f32r)
        nc.gpsimd.dma_start(out=b2_sb[:], in_=b2_h[:])
        # proj weights: prefetched once, used in phase C
        wp_sb = const.tile([128, D8, D], bf16)
        nc.gpsimd.dma_start(out=wp_sb[:], in_=wp_h[:])

        # AllToAll buffers: Ic[chunk, my-128-attn-rows, 512 toks] ->
        # Oc[dblock, 128 rows, my 512 toks]
        Ic = dram.tile([NCORES, 128, CHK], bf16)
        Oc = dram.tile([NCORES, 128, CHK], bf16)

        def one_pass():
            # ---------------- Phase A: LN1 + QKV(T) + attention -----------------
            with ExitStack() as A:
                wA = A.enter_context(tc.tile_pool(name="wA", bufs=1))
                wq_sb = wA.tile([128, D8, 128], bf16)
                nc.gpsimd.dma_start(out=wq_sb[:], in_=wq_h[:])
                wk_sb = wA.tile([128, D8, 128], bf16)
                nc.gpsimd.dma_start(out=wk_sb[:], in_=wk_h[:])
                wv_sb = wA.tile([128, D8, 128], bf16)
                nc.gpsimd.dma_start(out=wv_sb[:], in_=wv_h[:])

                QT = wA.tile([128, TOK], bf16)   # rows: (h2, hd)
                KT = wA.tile([128, TOK], bf16)
                V = wA.tile([128, TOK // 128, 2, HD + 1], bf16)  # tok-major V + ones col
                nc.vector.memset(V[:, :, :, HD:HD + 1], 1.0)

                lnp = A.enter_context(tc.tile_pool(name="lnp", bufs=3))
                hTp = A.enter_context(tc.tile_pool(name="hTp", bufs=2))
                vtp = A.enter_context(tc.tile_pool(name="vtp", bufs=2))
                ptp = A.enter_context(tc.tile_pool(name="ptp", bufs=8))
                aop = A.enter_context(tc.tile_pool(name="aop", bufs=3))
                smp = A.enter_context(tc.tile_pool(name="smp", bufs=4))
                ps_tr = A.enter_context(tc.tile_pool(name="ps_tr", bufs=2, space="PSUM"))
                ps_mm = A.enter_context(tc.tile_pool(name="ps_mm", bufs=2, space="PSUM"))
                ps_s = A.enter_context(tc.tile_pool(name="ps_s", bufs=2, space="PSUM"))
                ps_av = A.enter_context(tc.tile_pool(name="ps_av", bufs=2, space="PSUM"))

                for blk in range(NB):
                    # hT block, laid out [tok-part, t4, a, tok-in-tile] so the
                    # XBAR transpose writes a contiguous destination per t4.
                    hTb = hTp.tile([128, 4, D8, 128], bf16, tag="hTb")
                    for t4 in range(4):
                        tt = blk * 4 + t4
                        xt = lnp.tile([128, D], bf16, tag="xt")
                        nc.sync.dma_start(out=xt[:], in_=xbf_h[tt * 128:(tt + 1) * 128, :])
                        st = lnp.tile([128, 2, 6], f32, tag="st")
                        xv = xt[:].rearrange("p (s d) -> p s d", s=2)
                        nc.vector.bn_stats(out=st[:, 0, :], in_=xv[:, 0, :])
                        nc.vector.bn_stats(out=st[:, 1, :], in_=xv[:, 1, :])
                        mv = lnp.tile([128, 2], f32, tag="mv")
                        nc.vector.bn_aggr(out=mv[:], in_=st[:])
                        nc.scalar.activation(out=mv[:, 1:2], in_=mv[:, 1:2], func=AF.Sqrt,
                                             bias=eps_sb[:])
                        nc.vector.reciprocal(out=mv[:, 1:2], in_=mv[:, 1:2])
                        yt = lnp.tile([128, D], bf16, tag="yt")
                        nc.vector.tensor_scalar(out=yt[:], in0=xt[:],
                                                scalar1=mv[:, 0:1], scalar2=mv[:, 1:2],
                                                op0=ALU.subtract, op1=ALU.mult)
                        nc.sync.dma_start_transpose(out=hTb[:, t4], in_=yt[:])

                    for w_sb, b_sb, kind in ((wq_sb, bq_sb, "q"), (wk_sb, bk_sb, "k"),
                                             (wv_sb, bv_sb, "v")):
                        ps = ps_mm.tile([128, 512], f32, tag="qkvps")
                        for a in range(D8):
                            nc.tensor.matmul(ps[:], w_sb[:, a, :], hTb[:, :, a, :],
                                             start=(a == 0), stop=(a == D8 - 1))
                        if kind == "q":
                            nc.scalar.activation(out=QT[:, blk * 512:(blk + 1) * 512],
                                                 in_=ps[:], func=AF.Identity, bias=b_sb[:])
                        elif kind == "k":
                            nc.scalar.activation(out=KT[:, blk * 512:(blk + 1) * 512],
                                                 in_=ps[:], func=AF.Identity, bias=b_sb[:])
                        else:
                            vtmp = vtp.tile([128, 512], bf16, tag="vtmp")
                            nc.scalar.activation(out=vtmp[:], in_=ps[:],
                                                 func=AF.Identity, bias=b_sb[:])
                            tp2 = ps_tr.tile([128, 4, 128], bf16, tag="tp")
                            for q in range(4):
                                nc.tensor.transpose(tp2[:, q, :],
                                                    vtmp[:, q * 128:(q + 1) * 128], ident[:])
                            nc.vector.tensor_copy(
                                out=V[:, blk * 4:(blk + 1) * 4, :, 0:HD],
                                in_=tp2[:].rearrange("p q (h d) -> p q h d", h=2))

                # attention: 4 (batch, head) pairs
                for bh in range(B):
                    for h2 in range(2):
                        ro = h2 * HD
                        for j in range(4):          # query blocks of 512
                            q0 = bh * S + j * 512
                            nkt = 4 * (j + 1)
                            av = ps_av.tile([HD + 1, 512], f32, tag="av")
                            for kt in range(nkt):
                                k0 = bh * S + kt * 128
                                # straddling k-tiles: columns q < k0 are dead; only
                                # compute the live suffix and mask the one diagonal
                                # 128x128 sub-block.
                                ofs = max(0, 128 * kt - 512 * j)
                                w = 512 - ofs
                                sp = ps_s.tile([128, 512], f32, tag="sp")
                                nc.tensor.matmul(sp[:, 0:w], KT[ro:ro + HD, k0:k0 + 128],
                                                 QT[ro:ro + HD, q0 + ofs:q0 + 512],
                                                 start=True, stop=True)
                                if kt >= 4 * j:
                                    nc.vector.tensor_add(sp[:, 0:128], sp[:, 0:128],
                                                         mask[:])
                                pt = ptp.tile([128, 512], bf16, tag="pt")
                                nc.scalar.activation(out=pt[:, 0:w], in_=sp[:, 0:w],
                                                     func=AF.Exp, scale=SCALE)
                                g = bh * (S // 128) + kt
                                nc.tensor.matmul(av[:, ofs:512], V[:, g, h2, :], pt[:, 0:w],
                                                 start=(kt == 0), stop=(kt == nkt - 1))
                            den = smp.tile([1, 512], f32r, tag="den")
                            with nc.allow_low_precision(reason="f32r rounding of softmax denom"):
                                nc.vector.reciprocal(out=den[:], in_=av[HD:HD + 1, :])
                            bc = ps_s.tile([128, 512], f32, tag="sp")
                            nc.tensor.matmul(bc[0:HD, :], ones[:, 0:HD], den[:],
                                             start=True, stop=True)
                            rbc = aop.tile([HD, 512], f32, tag="rbc")
                            nc.scalar.activation(out=rbc[:], in_=bc[0:HD, :], func=AF.Copy)
                            ao = aop.tile([HD, 512], bf16, tag="ao")
                            nc.vector.tensor_mul(ao[:], av[0:HD, :], rbc[:])
                            chunk = bh * 4 + j
                            nc.sync.dma_start(out=Ic[chunk, h2 * HD:(h2 + 1) * HD, :],
                                              in_=ao[:])

            # ---------------- AllToAll: head-major -> token-major ---------------
            nc.gpsimd.collective_compute(
                "AllToAll", ALU.bypass,
                replica_groups=[list(range(NCORES))],
                ins=[Ic[:]], outs=[Oc[:]],
            )

            # ------------- Phase C: proj + residual + LN2 + MLP ------------------
            with ExitStack() as C:
                rB = C.enter_context(tc.tile_pool(name="rB", bufs=1))
                Oc_sb = rB.tile([128, NCORES, CHK], bf16)
                nc.sync.dma_start(out=Oc_sb[:], in_=Oc[:].rearrange("i p t -> p i t"))
                x2_sb = rB.tile([128, 4, D], f32)
                y2T = rB.tile([128, 4, D8, 128], bf16)
                g1T = rB.tile([128, NF, CHK], bf16)

                with ExitStack() as C1:
                    lnp2 = C1.enter_context(tc.tile_pool(name="lnp2", bufs=2))
                    w1s = C1.enter_context(tc.tile_pool(name="w1s", bufs=2))
                    ps_p = C1.enter_context(tc.tile_pool(name="ps_p", bufs=2, space="PSUM"))
                    ps_f1 = C1.enter_context(tc.tile_pool(name="ps_f1", bufs=2, space="PSUM"))

                    for t4 in range(4):
                        xct = lnp2.tile([128, D], f32, tag="xct")
                        nc.sync.dma_start(out=xct[:], in_=xc_h[t4 * 128:(t4 + 1) * 128, :])
                        for dc in range(2):
                            ps = ps_p.tile([128, 512], f32, tag="pp")
                            for a in range(D8):
                                nc.tensor.matmul(ps[:], Oc_sb[:, a, t4 * 128:(t4 + 1) * 128],
                                                 wp_sb[:, a, dc * 512:(dc + 1) * 512],
                                                 start=(a == 0), stop=False)
                            nc.tensor.matmul(ps[:], ones[:, 0:128],
                                             bp_sb[:, dc * 512:(dc + 1) * 512],
                                             start=False, stop=True)
                            nc.vector.tensor_add(x2_sb[:, t4, dc * 512:(dc + 1) * 512], ps[:],
                                                 xct[:, dc * 512:(dc + 1) * 512])
                        st2 = lnp2.tile([128, 2, 6], f32, tag="st2")
                        x2v = x2_sb[:, t4, :].rearrange("p (s d) -> p s d", s=2)
                        nc.vector.bn_stats(out=st2[:, 0, :], in_=x2v[:, 0, :])
                        nc.vector.bn_stats(out=st2[:, 1, :], in_=x2v[:, 1, :])
                        mv2 = lnp2.tile([128, 2], f32, tag="mv2")
                        nc.vector.bn_aggr(out=mv2[:], in_=st2[:])
                        nc.scalar.activation(out=mv2[:, 1:2], in_=mv2[:, 1:2], func=AF.Sqrt,
                                             bias=eps_sb[:])
                        nc.vector.reciprocal(out=mv2[:, 1:2], in_=mv2[:, 1:2])
                        y2 = lnp2.tile([128, D], bf16, tag="y2")
                        nc.vector.tensor_scalar(out=y2[:], in0=x2_sb[:, t4, :],
                                                scalar1=mv2[:, 0:1], scalar2=mv2[:, 1:2],
                                                op0=ALU.subtract, op1=ALU.mult)
                        nc.sync.dma_start_transpose(out=y2T[:, t4], in_=y2[:])

                    # fc1 + gelu -> g1T resident
                    for ff in range(NF):
                        w1t = w1s.tile([128, D8, 128], bf16, tag="w1t")
                        nc.gpsimd.dma_start(out=w1t[:], in_=w1_h[:, ff])
                        ps = ps_f1.tile([128, 512], f32, tag="f1")
                        for a in range(D8):
                            nc.tensor.matmul(ps[:], w1t[:, a, :], y2T[:, :, a, :],
                                             start=(a == 0), stop=(a == D8 - 1))
                        nc.scalar.activation(out=g1T[:, ff, :], in_=ps[:], func=AF.Gelu,
                                             bias=b1_sb[:, ff:ff + 1])

                # fc2: 8 psum accumulators (4 token tiles x 2 column halves)
                ps_f2 = C.enter_context(tc.tile_pool(name="ps_f2", bufs=1, space="PSUM"))
                w2s = C.enter_context(tc.tile_pool(name="w2s", bufs=3))
                outp = C.enter_context(tc.tile_pool(name="outp", bufs=2))
                accs = [ps_f2.tile([128, 512], f32, name=f"acc{i}", tag=f"acc{i}")
                        for i in range(8)]
                for ff in range(NF):
                    w2t = w2s.tile([128, D], bf16, tag="w2t")
                    nc.gpsimd.dma_start(out=w2t[:], in_=w2_h[ff * 128:(ff + 1) * 128, :])
                    for t4 in range(4):
                        for dc in range(2):
                            nc.tensor.matmul(accs[t4 * 2 + dc][:],
                                             g1T[:, ff, t4 * 128:(t4 + 1) * 128],
                                             w2t[:, dc * 512:(dc + 1) * 512],
                                             start=(ff == 0), stop=False)
                for t4 in range(4):
                    ot = outp.tile([128, D], f32, tag="ot")
                    for dc in range(2):
                        i = t4 * 2 + dc
                        nc.tensor.matmul(accs[i][:], ones[:, 0:128],
                                         b2_sb[:, dc * 512:(dc + 1) * 512],
                                         start=False, stop=True)
                        nc.vector.tensor_add(ot[:, dc * 512:(dc + 1) * 512], accs[i][:],
                                             x2_sb[:, t4, dc * 512:(dc + 1) * 512])
                    nc.sync.dma_start(out=out_h[t4 * 128:(t4 + 1) * 128, :], in_=ot[:])


        for _ in range(loops):
            one_pass()
    nc.finalize()
    return nc


_NC_CACHE = []
LAST = None


def _get_nc():
    if not _NC_CACHE:
        _NC_CACHE.append(build())
    return _NC_CACHE[0]


def prepare_in_maps(inputs):
    f = np.float32
    x = np.ascontiguousarray(np.asarray(inputs["x"], f).reshape(TOK, D))
    xbf = x.astype(NPBF)
    ln1_g = np.asarray(inputs["ln1_g"], np.float64)
    ln1_b = np.asarray(inputs["ln1_b"], np.float64)
    ln2_g = np.asarray(inputs["ln2_g"], np.float64)
    ln2_b = np.asarray(inputs["ln2_b"], np.float64)
    w_qkv = np.asarray(inputs["w_qkv"], np.float64)
    b_qkv = np.asarray(inputs["b_qkv"], np.float64)
    w_fc1 = np.asarray(inputs["w_fc1"], np.float64)
    b_fc1 = np.asarray(inputs["b_fc1"], np.float64)

    w_eff = (w_qkv * ln1_g[:, None]).astype(f)
    b_eff = (b_qkv + ln1_b @ w_qkv).astype(f)
    w1_eff = (w_fc1 * ln2_g[:, None]).astype(f)
    b1_eff = (b_fc1 + ln2_b @ w_fc1).astype(f)
    wp = np.asarray(inputs["w_proj"], f)
    bp = np.asarray(inputs["b_proj"], f).reshape(1, D)
    w2 = np.asarray(inputs["w_fc2"], f)
    b2 = np.asarray(inputs["b_fc2"], f).reshape(1, D)

    # weight layouts pre-arranged on the host so every device DMA is contiguous:
    # w_re[p, a, c] = w[a*128 + p, c]
    def contract_tiles(w):  # [D, N] -> [128, D8, N]
        return np.ascontiguousarray(
            w.reshape(D8, 128, w.shape[1]).transpose(1, 0, 2)).astype(NPBF)

    wp_re = contract_tiles(wp)
    w1_re = np.ascontiguousarray(
        w1_eff.reshape(D8, 128, NF, 128).transpose(1, 2, 0, 3)).astype(NPBF)
    b1c = np.ascontiguousarray(b1_eff.reshape(NF, 128).T).astype(f)
    w2_bf = np.ascontiguousarray(w2).astype(NPBF)

    # additive causal mask for the diagonal 128x128 sub-block:
    # mask[i, j] = 0 where query j >= key i, else -1e9 (-> exp == 0)
    mask = np.full((128, 128), -1e9, f)
    for i in range(128):
        mask[i, i:] = 0.0
    ident = np.eye(128, dtype=NPBF)

    in_maps = []
    for c in range(NCORES):
        cs = slice(128 * c, 128 * (c + 1))
        in_maps.append({
            "xbf": xbf,
            "xc": np.ascontiguousarray(x[CHK * c:CHK * (c + 1)]),
            "wq": contract_tiles(w_eff[:, 0 * D:1 * D][:, cs]),
            "wk": contract_tiles(w_eff[:, 1 * D:2 * D][:, cs]),
            "wv": contract_tiles(w_eff[:, 2 * D:3 * D][:, cs]),
            "bq": np.ascontiguousarray(b_eff[0 * D:1 * D][cs].reshape(128, 1)),
            "bk": np.ascontiguousarray(b_eff[1 * D:2 * D][cs].reshape(128, 1)),
            "bv": np.ascontiguousarray(b_eff[2 * D:3 * D][cs].reshape(128, 1)),
            "wp": wp_re, "bp": bp,
            "w1": w1_re, "b1": b1c,
            "w2": w2_bf, "b2": b2,
            "mask": mask, "ident": ident,
            "ones": np.ones((1, 512), f),
        })
    return in_maps


def kernel(**inputs):
    global LAST
    in_maps = prepare_in_maps(inputs)
    nc = _get_nc()
    res = run_bass_kernel_spmd(nc, in_maps, list(range(NCORES)))
    LAST = res
    out = np.concatenate([res.results[c]["out"] for c in range(NCORES)], axis=0)
    return out.reshape(B, S, D).astype(np.float32, copy=False)


# revision 10
# speedup vs baseline: 1.9705x; 1.1583x over previous
"""Causal transformer block (B=2,S=2048,D=1024,H=16) on 8 trn2 NeuronCores.

Strategy: tensor-parallel attention over heads (2 heads/core) + token-parallel
MLP (512 tokens/core), glued by a single small AllToAll (1MB/core, bf16) that
re-shards the attention output from head-major to token-major.  LayerNorm
gains/biases are folded into the following matmul weights on the host; QKV /
fc1 biases are folded into the PSUM->SBUF copy as per-partition activation
biases; softmax skips the max-subtraction (scores are bounded) and gets its
denominator from a ones-column appended to V.  All matmul operands are bf16
(f32 PSUM accumulation); the residual spine stays f32.  Activation transposes
(x -> hT, y2 -> y2T) run on the DMA XBAR (dma_start_transpose) instead of the
PE array; V's transpose stays on the PE because its destination interleaves
the ones column.
"""
import numpy as np
import ml_dtypes
from contextlib import ExitStack

import concourse.bass as bass
import concourse.bacc as bacc
import concourse.tile as tile
from concourse import mybir
from concourse.bass_utils import run_bass_kernel_spmd

f32 = mybir.dt.float32
f32r = mybir.dt.float32r
bf16 = mybir.dt.bfloat16
AF = mybir.ActivationFunctionType
ALU = mybir.AluOpType

B, S, D, H, HD, FF, NCORES = 2, 2048, 1024, 16, 64, 4096, 8
TOK = B * S            # 4096 total tokens
CHK = TOK // NCORES    # 512 tokens per core
D8 = D // 128          # 8 contraction tiles over D
NF = FF // 128         # 32 tiles over FF
NB = TOK // 512        # 8 token blocks of 512
EPS = 1e-5
SCALE = 1.0 / float(np.sqrt(HD))
NPBF = ml_dtypes.bfloat16


def build(loops=1):
    nc = bacc.Bacc(None, num_devices=NCORES)

    xbf_h = nc.declare_dram_parameter("xbf", [TOK, D], bf16, isOutput=False)
    xc_h = nc.declare_dram_parameter("xc", [CHK, D], f32, isOutput=False)
    wq_h = nc.declare_dram_parameter("wq", [128, D8, 128], bf16, isOutput=False)
    wk_h = nc.declare_dram_parameter("wk", [128, D8, 128], bf16, isOutput=False)
    wv_h = nc.declare_dram_parameter("wv", [128, D8, 128], bf16, isOutput=False)
    bq_h = nc.declare_dram_parameter("bq", [128, 1], f32, isOutput=False)
    bk_h = nc.declare_dram_parameter("bk", [128, 1], f32, isOutput=False)
    bv_h = nc.declare_dram_parameter("bv", [128, 1], f32, isOutput=False)
    wp_h = nc.declare_dram_parameter("wp", [128, D8, D], bf16, isOutput=False)
    bp_h = nc.declare_dram_parameter("bp", [1, D], f32r, isOutput=False)
    w1_h = nc.declare_dram_parameter("w1", [128, NF, D8, 128], bf16, isOutput=False)
    b1_h = nc.declare_dram_parameter("b1", [128, NF], f32, isOutput=False)
    w2_h = nc.declare_dram_parameter("w2", [FF, D], bf16, isOutput=False)
    b2_h = nc.declare_dram_parameter("b2", [1, D], f32r, isOutput=False)
    mask_h = nc.declare_dram_parameter("mask", [128, 128], f32, isOutput=False)
    ones_h = nc.declare_dram_parameter("ones", [1, 512], f32r, isOutput=False)
    id_h = nc.declare_dram_parameter("ident", [128, 128], bf16, isOutput=False)
    out_h = nc.declare_dram_parameter("out", [CHK, D], f32, isOutput=True)

    with tile.TileContext(nc) as tc, ExitStack() as top:
        const = top.enter_context(tc.tile_pool(name="const", bufs=1))
        dram = top.enter_context(tc.tile_pool(name="dramp", bufs=1, space="DRAM"))

        ident = const.tile([128, 128], bf16)
        nc.gpsimd.dma_start(out=ident[:], in_=id_h[:])
        mask = const.tile([128, 128], f32)
        nc.gpsimd.dma_start(out=mask[:], in_=mask_h[:])
        eps_sb = const.tile([128, 1], f32)
        nc.vector.memset(eps_sb[:], EPS)
        ones = const.tile([1, 512], f32r)
        nc.gpsimd.dma_start(out=ones[:], in_=ones_h[:])
        bq_sb = const.tile([128, 1], f32)
        nc.gpsimd.dma_start(out=bq_sb[:], in_=bq_h[:])
        bk_sb = const.tile([128, 1], f32)
        nc.gpsimd.dma_start(out=bk_sb[:], in_=bk_h[:])
        bv_sb = const.tile([128, 1], f32)
        nc.gpsimd.dma_start(out=bv_sb[:], in_=bv_h[:])
        bp_sb = const.tile([1, D], f32r)
        nc.gpsimd.dma_start(out=bp_sb[:], in_=bp_h[:])
        b1_sb = const.tile([128, NF], f32)
        nc.gpsimd.dma_start(out=b1_sb[:], in_=b1_h[:])
        b2_sb = const.tile([1, D], f32r)
        nc.gpsimd.dma_start(out=b2_sb[:], in_=b2_h[:])
        # proj weights: prefetched once, used in phase C
        wp_sb = const.tile([128, D8, D], bf16)
        nc.gpsimd.dma_start(out=wp_sb[:], in_=wp_h[:])

        # AllToAll buffers: Ic[chunk, my-128-attn-rows, 512 toks] ->
        # Oc[dblock, 128 rows, my 512 toks]
        Ic = dram.tile([NCORES, 128, CHK], bf16)
        Oc = dram.tile([NCORES, 128, CHK], bf16)

        def one_pass():
            # ---------------- Phase A: LN1 + QKV(T) + attention -----------------
            with ExitStack() as A:
                wA = A.enter_context(tc.tile_pool(name="wA", bufs=1))
                wq_sb = wA.tile([128, D8, 128], bf16)
                nc.gpsimd.dma_start(out=wq_sb[:], in_=wq_h[:])
                wk_sb = wA.tile([128, D8, 128], bf16)
                nc.gpsimd.dma_start(out=wk_sb[:], in_=wk_h[:])
                wv_sb = wA.tile([128, D8, 128], bf16)
                nc.gpsimd.dma_start(out=wv_sb[:], in_=wv_h[:])

                QT = wA.tile([128, TOK], bf16)   # rows: (h2, hd)
                KT = wA.tile([128, TOK], bf16)
                V = wA.tile([128, TOK // 128, 2, HD + 1], bf16)  # tok-major V + ones col
                nc.vector.memset(V[:, :, :, HD:HD + 1], 1.0)

                lnp = A.enter_context(tc.tile_pool(name="lnp", bufs=3))
                hTp = A.enter_context(tc.tile_pool(name="hTp", bufs=2))
                vtp = A.enter_context(tc.tile_pool(name="vtp", bufs=2))
                ptp = A.enter_context(tc.tile_pool(name="ptp", bufs=8))
                aop = A.enter_context(tc.tile_pool(name="aop", bufs=3))
                smp = A.enter_context(tc.tile_pool(name="smp", bufs=4))
                ps_tr = A.enter_context(tc.tile_pool(name="ps_tr", bufs=1, space="PSUM"))
                ps_mm = A.enter_context(tc.tile_pool(name="ps_mm", bufs=2, space="PSUM"))
                ps_s = A.enter_context(tc.tile_pool(name="ps_s", bufs=3, space="PSUM"))
                ps_av = A.enter_context(tc.tile_pool(name="ps_av", bufs=1, space="PSUM"))

                for blk in range(NB):
                    # hT block, laid out [tok-part, t4, a, tok-in-tile] so the
                    # XBAR transpose writes a contiguous destination.
                    hTb = hTp.tile([128, 4, D8, 128], bf16, tag="hTb")
                    ybt = lnp.tile([128, 4, D], bf16, tag="ybt")
                    for t4 in range(4):
                        tt = blk * 4 + t4
                        if t4 % 2 == 0:
                            xt2 = lnp.tile([128, 2, D], bf16, tag="xt")
                            nc.sync.dma_start(
                                out=xt2[:],
                                in_=xbf_h[tt * 128:(tt + 2) * 128, :].rearrange(
                                    "(b p) d -> p b d", p=128))
                        xt = xt2[:, t4 % 2, :]
                        st = lnp.tile([128, 2, 6], f32, tag="st")
                        xv = xt.rearrange("p (s d) -> p s d", s=2)
                        nc.vector.bn_stats(out=st[:, 0, :], in_=xv[:, 0, :])
                        nc.vector.bn_stats(out=st[:, 1, :], in_=xv[:, 1, :])
                        mv = lnp.tile([128, 2], f32, tag="mv")
                        nc.vector.bn_aggr(out=mv[:], in_=st[:])
                        # rstd = exp(-0.5*ln(var+eps)): Ln+Exp share one scalar
                        # activation table with the attention Exp (Sqrt doesn't).
                        nc.scalar.activation(out=mv[:, 1:2], in_=mv[:, 1:2], func=AF.Ln,
                                             bias=eps_sb[:])
                        nc.scalar.activation(out=mv[:, 1:2], in_=mv[:, 1:2], func=AF.Exp,
                                             scale=-0.5)
                        nc.vector.tensor_scalar(out=ybt[:, t4, :], in0=xt,
                                                scalar1=mv[:, 0:1], scalar2=mv[:, 1:2],
                                                op0=ALU.subtract, op1=ALU.mult)
                    nc.sync.dma_start_transpose(out=hTb[:], in_=ybt[:])

                    for w_sb, b_sb, kind in ((wq_sb, bq_sb, "q"), (wk_sb, bk_sb, "k"),
                                             (wv_sb, bv_sb, "v")):
                        ps = ps_mm.tile([128, 512], f32, tag="qkvps")
                        for a in range(D8):
                            nc.tensor.matmul(ps[:], w_sb[:, a, :], hTb[:, :, a, :],
                                             start=(a == 0), stop=(a == D8 - 1))
                        if kind == "q":
                            nc.scalar.activation(out=QT[:, blk * 512:(blk + 1) * 512],
                                                 in_=ps[:], func=AF.Identity, bias=b_sb[:])
                        elif kind == "k":
                            nc.scalar.activation(out=KT[:, blk * 512:(blk + 1) * 512],
                                                 in_=ps[:], func=AF.Identity, bias=b_sb[:])
                        else:
                            vtmp = vtp.tile([128, 512], bf16, tag="vtmp")
                            nc.scalar.activation(out=vtmp[:], in_=ps[:],
                                                 func=AF.Identity, bias=b_sb[:])
                            tp2 = ps_tr.tile([128, 4, 128], bf16, tag="tp2")
                            for q in range(4):
                                nc.tensor.transpose(tp2[:, q, :],
                                                    vtmp[:, q * 128:(q + 1) * 128], ident[:])
                            nc.vector.tensor_copy(
                                out=V[:, blk * 4:(blk + 1) * 4, :, 0:HD],
                                in_=tp2[:].rearrange("p q (h d) -> p q h d", h=2))

                # attention: per (batch, query-block), the two heads' kt-chains
                # are interleaved so the PE always has an independent matmul
                # ready while the other head's scores sit in the exp stage.
                for bh in range(B):
                    for j in range(4):          # query blocks of 512
                        q0 = bh * S + j * 512
                        nkt = 4 * (j + 1)
                        avs = [ps_av.tile([HD + 1, 512], f32, name=f"av{h2}",
                                          tag=f"av{h2}")
                               for h2 in range(2)]
                        for kt in range(nkt):
                            k0 = bh * S + kt * 128
                            # straddling k-tiles: columns q < k0 are dead; only
                            # compute the live suffix and mask the one diagonal
                            # 128x128 sub-block.
                            ofs = max(0, 128 * kt - 512 * j)
                            w = 512 - ofs
                            g = bh * (S // 128) + kt
                            for h2 in range(2):
                                ro = h2 * HD
                                sp = ps_s.tile([128, 512], f32, tag="sp")
                                nc.tensor.matmul(sp[:, 0:w], KT[ro:ro + HD, k0:k0 + 128],
                                                 QT[ro:ro + HD, q0 + ofs:q0 + 512],
                                                 start=True, stop=True)
                                if kt >= 4 * j:
                                    nc.vector.tensor_add(sp[:, 0:128], sp[:, 0:128],
                                                         mask[:])
                                pt = ptp.tile([128, 512], bf16, tag="pt")
                                nc.scalar.activation(out=pt[:, 0:w], in_=sp[:, 0:w],
                                                     func=AF.Exp, scale=SCALE)
                                nc.tensor.matmul(avs[h2][:, ofs:512], V[:, g, h2, :],
                                                 pt[:, 0:w],
                                                 start=(kt == 0), stop=(kt == nkt - 1))
                        for h2 in range(2):
                            av = avs[h2]
                            den = smp.tile([1, 512], f32r, tag="den")
                            nc.vector.tensor_copy(out=den[:], in_=av[HD:HD + 1, :])
                            bc = ps_s.tile([128, 512], f32, tag="sp")
                            nc.tensor.matmul(bc[0:HD, :], ones[:, 0:HD], den[:],
                                             start=True, stop=True)
                            # reciprocal on the broadcast block: all 64 lanes work,
                            # vs ~3.4us for an iterative reciprocal on one lane.
                            rbc = aop.tile([HD, 512], f32, tag="rbc")
                            nc.vector.reciprocal_approx_fast(out=rbc[:], in_=bc[0:HD, :])
                            ao = aop.tile([HD, 512], bf16, tag="ao")
                            nc.vector.tensor_mul(ao[:], av[0:HD, :], rbc[:])
                            chunk = bh * 4 + j
                            nc.sync.dma_start(out=Ic[chunk, h2 * HD:(h2 + 1) * HD, :],
                                              in_=ao[:])

            # ---------------- AllToAll: head-major -> token-major ---------------
            nc.gpsimd.collective_compute(
                "AllToAll", ALU.bypass,
                replica_groups=[list(range(NCORES))],
                ins=[Ic[:]], outs=[Oc[:]],
            )

            # ------------- Phase C: proj + residual + LN2 + MLP ------------------
            with ExitStack() as C:
                rB = C.enter_context(tc.tile_pool(name="rB", bufs=1))
                Oc_sb = rB.tile([128, NCORES, CHK], bf16)
                nc.sync.dma_start(out=Oc_sb[:], in_=Oc[:].rearrange("i p t -> p i t"))
                x2_sb = rB.tile([128, 4, D], f32)
                y2T = rB.tile([128, 4, D8, 128], bf16)
                g1T = rB.tile([128, NF, CHK], bf16)

                with ExitStack() as C1:
                    lnp2 = C1.enter_context(tc.tile_pool(name="lnp2", bufs=2))
                    w1s = C1.enter_context(tc.tile_pool(name="w1s", bufs=2))
                    ps_p = C1.enter_context(tc.tile_pool(name="ps_p", bufs=2, space="PSUM"))
                    ps_f1 = C1.enter_context(tc.tile_pool(name="ps_f1", bufs=2, space="PSUM"))

                    for t4 in range(4):
                        xct = lnp2.tile([128, D], f32, tag="xct")
                        nc.sync.dma_start(out=xct[:], in_=xc_h[t4 * 128:(t4 + 1) * 128, :])
                        for dc in range(2):
                            ps = ps_p.tile([128, 512], f32, tag="pp")
                            for a in range(D8):
                                nc.tensor.matmul(ps[:], Oc_sb[:, a, t4 * 128:(t4 + 1) * 128],
                                                 wp_sb[:, a, dc * 512:(dc + 1) * 512],
                                                 start=(a == 0), stop=False)
                            nc.tensor.matmul(ps[:], ones[:, 0:128],
                                             bp_sb[:, dc * 512:(dc + 1) * 512],
                                             start=False, stop=True)
                            nc.vector.tensor_add(x2_sb[:, t4, dc * 512:(dc + 1) * 512], ps[:],
                                                 xct[:, dc * 512:(dc + 1) * 512])
                        st2 = lnp2.tile([128, 2, 6], f32, tag="st2")
                        x2v = x2_sb[:, t4, :].rearrange("p (s d) -> p s d", s=2)
                        nc.vector.bn_stats(out=st2[:, 0, :], in_=x2v[:, 0, :])
                        nc.vector.bn_stats(out=st2[:, 1, :], in_=x2v[:, 1, :])
                        mv2 = lnp2.tile([128, 2], f32, tag="mv2")
                        nc.vector.bn_aggr(out=mv2[:], in_=st2[:])
                        nc.scalar.activation(out=mv2[:, 1:2], in_=mv2[:, 1:2], func=AF.Ln,
                                             bias=eps_sb[:])
                        nc.scalar.activation(out=mv2[:, 1:2], in_=mv2[:, 1:2], func=AF.Exp,
                                             scale=-0.5)
                        y2 = lnp2.tile([128, D], bf16, tag="y2")
                        nc.vector.tensor_scalar(out=y2[:], in0=x2_sb[:, t4, :],
                                                scalar1=mv2[:, 0:1], scalar2=mv2[:, 1:2],
                                                op0=ALU.subtract, op1=ALU.mult)
                        nc.sync.dma_start_transpose(out=y2T[:, t4], in_=y2[:])

                    # fc1 + gelu -> g1T resident
                    for ff in range(NF):
                        w1t = w1s.tile([128, D8, 128], bf16, tag="w1t")
                        nc.gpsimd.dma_start(out=w1t[:], in_=w1_h[:, ff])
                        ps = ps_f1.tile([128, 512], f32, tag="f1")
                        for a in range(D8):
                            nc.tensor.matmul(ps[:], w1t[:, a, :], y2T[:, :, a, :],
                                             start=(a == 0), stop=(a == D8 - 1))
                        nc.scalar.activation(out=g1T[:, ff, :], in_=ps[:], func=AF.Gelu,
                                             bias=b1_sb[:, ff:ff + 1])

                # fc2: 8 psum accumulators (4 token tiles x 2 column halves)
                ps_f2 = C.enter_context(tc.tile_pool(name="ps_f2", bufs=1, space="PSUM"))
                w2s = C.enter_context(tc.tile_pool(name="w2s", bufs=3))
                outp = C.enter_context(tc.tile_pool(name="outp", bufs=2))
                accs = [ps_f2.tile([128, 512], f32, name=f"acc{i}", tag=f"acc{i}")
                        for i in range(8)]
                for ff in range(NF):
                    w2t = w2s.tile([128, D], bf16, tag="w2t")
                    nc.gpsimd.dma_start(out=w2t[:], in_=w2_h[ff * 128:(ff + 1) * 128, :])
                    for t4 in range(4):
                        for dc in range(2):
                            nc.tensor.matmul(accs[t4 * 2 + dc][:],
                                             g1T[:, ff, t4 * 128:(t4 + 1) * 128],
                                             w2t[:, dc * 512:(dc + 1) * 512],
                                             start=(ff == 0), stop=False)
                for t4 in range(4):
                    ot = outp.tile([128, D], f32, tag="ot")
                    for dc in range(2):
                        i = t4 * 2 + dc
                        nc.tensor.matmul(accs[i][:], ones[:, 0:128],
                                         b2_sb[:, dc * 512:(dc + 1) * 512],
                                         start=False, stop=True)
                        nc.vector.tensor_add(ot[:, dc * 512:(dc + 1) * 512], accs[i][:],
                                             x2_sb[:, t4, dc * 512:(dc + 1) * 512])
                    nc.sync.dma_start(out=out_h[t4 * 128:(t4 + 1) * 128, :], in_=ot[:])


        for _ in range(loops):
            one_pass()
    nc.finalize()
    return nc


_NC_CACHE = []
LAST = None


def _get_nc():
    if not _NC_CACHE:
        _NC_CACHE.append(build())
    return _NC_CACHE[0]


def prepare_in_maps(inputs):
    f = np.float32
    x = np.ascontiguousarray(np.asarray(inputs["x"], f).reshape(TOK, D))
    xbf = x.astype(NPBF)
    ln1_g = np.asarray(inputs["ln1_g"], np.float64)
    ln1_b = np.asarray(inputs["ln1_b"], np.float64)
    ln2_g = np.asarray(inputs["ln2_g"], np.float64)
    ln2_b = np.asarray(inputs["ln2_b"], np.float64)
    w_qkv = np.asarray(inputs["w_qkv"], np.float64)
    b_qkv = np.asarray(inputs["b_qkv"], np.float64)
    w_fc1 = np.asarray(inputs["w_fc1"], np.float64)
    b_fc1 = np.asarray(inputs["b_fc1"], np.float64)

    w_eff = (w_qkv * ln1_g[:, None]).astype(f)
    b_eff = (b_qkv + ln1_b @ w_qkv).astype(f)
    w1_eff = (w_fc1 * ln2_g[:, None]).astype(f)
    b1_eff = (b_fc1 + ln2_b @ w_fc1).astype(f)
    wp = np.asarray(inputs["w_proj"], f)
    bp = np.asarray(inputs["b_proj"], f).reshape(1, D)
    w2 = np.asarray(inputs["w_fc2"], f)
    b2 = np.asarray(inputs["b_fc2"], f).reshape(1, D)

    # weight layouts pre-arranged on the host so every device DMA is contiguous:
    # w_re[p, a, c] = w[a*128 + p, c]
    def contract_tiles(w):  # [D, N] -> [128, D8, N]
        return np.ascontiguousarray(
            w.reshape(D8, 128, w.shape[1]).transpose(1, 0, 2)).astype(NPBF)

    wp_re = contract_tiles(wp)
    w1_re = np.ascontiguousarray(
        w1_eff.reshape(D8, 128, NF, 128).transpose(1, 2, 0, 3)).astype(NPBF)
    b1c = np.ascontiguousarray(b1_eff.reshape(NF, 128).T).astype(f)
    w2_bf = np.ascontiguousarray(w2).astype(NPBF)

    # additive causal mask for the diagonal 128x128 sub-block:
    # mask[i, j] = 0 where query j >= key i, else -1e9 (-> exp == 0)
    mask = np.full((128, 128), -1e9, f)
    for i in range(128):
        mask[i, i:] = 0.0
    ident = np.eye(128, dtype=NPBF)

    in_maps = []
    for c in range(NCORES):
        cs = slice(128 * c, 128 * (c + 1))
        in_maps.append({
            "xbf": xbf,
            "xc": np.ascontiguousarray(x[CHK * c:CHK * (c + 1)]),
            "wq": contract_tiles(w_eff[:, 0 * D:1 * D][:, cs]),
            "wk": contract_tiles(w_eff[:, 1 * D:2 * D][:, cs]),
            "wv": contract_tiles(w_eff[:, 2 * D:3 * D][:, cs]),
            "bq": np.ascontiguousarray(b_eff[0 * D:1 * D][cs].reshape(128, 1)),
            "bk": np.ascontiguousarray(b_eff[1 * D:2 * D][cs].reshape(128, 1)),
            "bv": np.ascontiguousarray(b_eff[2 * D:3 * D][cs].reshape(128, 1)),
            "wp": wp_re, "bp": bp,
            "w1": w1_re, "b1": b1c,
            "w2": w2_bf, "b2": b2,
            "mask": mask, "ident": ident,
            "ones": np.ones((1, 512), f),
        })
    return in_maps


def kernel(**inputs):
    global LAST
    in_maps = prepare_in_maps(inputs)
    nc = _get_nc()
    res = run_bass_kernel_spmd(nc, in_maps, list(range(NCORES)))
    LAST = res
    out = np.concatenate([res.results[c]["out"] for c in range(NCORES)], axis=0)
    return out.reshape(B, S, D).astype(np.float32, copy=False)
